# revision 10
# baseline (speedup 1.0000x reference)
"""GAT attention head (nn_AttHead) on 8 Trainium2 NeuronCores.

Reference computation:
    h = input @ W                  [N, F]
    e_ij = leakyrelu(f_src_i + f_dst_j, 0.2);  masked softmax over j (mask=adj)
    h' = elu(softmax(e) @ h)

Key restructuring used here (exact algebra, not an approximation):
    exp(lrelu(s)) = exp(0.2 s) * max(exp(0.8 s), 1)
    s_ij = f_src_i + f_dst_j is rank-1, so with
        u_i = exp(0.8 f_src_i), v_j = exp(0.8 f_dst_j), q_j = exp(0.2 f_dst_j)
    the row factor exp(0.2 f_src_i) cancels in the softmax and
        att_ij ∝ A_ij * q_j * max(u_i v_j, 1)
        h'_i = (Σ_j A_ij max(u_i v_j,1) [q_j h_j, q_j]) / (denominator column)
    This removes every transcendental from the O(N^2) inner loop: per tile the
    device only needs one tensor_scalar (mult+max), one tensor_tensor (mask
    multiply) and a matmul accumulation; u, v, q are O(N) host precomputes.

Sharding: row-parallel over the N=8192 output rows; core c owns rows
[c*1024, (c+1)*1024). Scores are built transposed ([j on partitions, i free])
so the PE can contract over j directly. The adjacency mask is shipped as a
bf16 {0,1} matrix transposed to [j, i] layout (host-side data marshaling).
"""

import numpy as np
import ml_dtypes

N = 8192
IN_F = 128
OUT_F = 64
HT_F = OUT_F + 1  # h-tilde carries a denominator ones-column (scaled by q)
N_CORES = 8
SLAB = N // N_CORES  # 1024 output rows per core
P = 128
NT = N // P  # 64 j-chunks of 128
HALF = SLAB // 2  # PSUM free-dim limit for fp32 output is 512

_bf16 = ml_dtypes.bfloat16

_nc_cache = None


def _build_bass():
    import concourse.mybir as mybir
    import concourse.tile as tile
    from concourse import bacc

    bf = mybir.dt.bfloat16
    f32 = mybir.dt.float32
    Alu = mybir.AluOpType

    nc = bacc.Bacc("TRN2", target_bir_lowering=False, debug=False)

    maskT = nc.dram_tensor("maskT", [N, SLAB], bf, kind="ExternalInput")
    u_bc = nc.dram_tensor("u_bc", [P, SLAB], bf, kind="ExternalInput")
    vT = nc.dram_tensor("vT", [P, NT], f32, kind="ExternalInput")
    ht = nc.dram_tensor("ht", [P, NT * HT_F], bf, kind="ExternalInput")
    out = nc.dram_tensor("out", [OUT_F, SLAB], f32, kind="ExternalOutput")

    maskT_t = maskT.rearrange("(t p) i -> t p i", p=P)

    with tile.TileContext(nc) as tc:
        with (
            tc.tile_pool(name="const", bufs=1) as cpool,
            tc.tile_pool(name="mask", bufs=8) as mpool,
            tc.tile_pool(name="gt", bufs=4) as gpool,
            tc.tile_pool(name="pt", bufs=4) as ppool,
            tc.tile_pool(name="ps", bufs=1, space="PSUM") as pspool,
            tc.tile_pool(name="epi", bufs=1) as epool,
        ):
            u_sb = cpool.tile([P, SLAB], bf)
            nc.sync.dma_start(u_sb[:], u_bc[:])
            vT_sb = cpool.tile([P, NT], f32)
            nc.sync.dma_start(vT_sb[:], vT[:])
            ht_sb = cpool.tile([P, NT, HT_F], bf)
            nc.sync.dma_start(ht_sb[:], ht.rearrange("p (t f) -> p t f", f=HT_F))

            # Warm the ACT exp table during the main loop (ScalarE is idle);
            # output is unused.
            warm = cpool.tile([P, 8], f32)
            nc.scalar.activation(
                warm[:], u_sb[:, 0:8], mybir.ActivationFunctionType.Exp
            )

            ps0 = pspool.tile([HT_F, HALF], f32)
            ps1 = pspool.tile([HT_F, HALF], f32)

            # two j-chunks per group: one 512KB DMA, 2 tensor_scalar, one
            # [128, 2048] tensor_tensor (alternating DVE / GpSimd), 4 matmuls
            for tt in range(NT // 2):
                m2 = mpool.tile([P, 2, SLAB], bf)
                nc.sync.dma_start(
                    m2[:], maskT_t[2 * tt : 2 * tt + 2].rearrange("t p i -> p t i")
                )
                g2 = gpool.tile([P, 2, SLAB], bf)
                for b in range(2):
                    t = 2 * tt + b
                    # g = max(u_i * v_j, 1)
                    nc.vector.tensor_scalar(
                        g2[:, b, :], u_sb[:], vT_sb[:, t : t + 1], 1.0,
                        Alu.mult, Alu.max,
                    )
                # p = g * mask  (alternate DVE / GpSimd to split the load)
                p2 = ppool.tile([P, 2, SLAB], bf)
                eng = nc.gpsimd if tt % 2 == 0 else nc.vector
                eng.tensor_tensor(p2[:], g2[:], m2[:], Alu.mult)
                # accumulate [65, i] += htil_chunk.T @ p
                for b in range(2):
                    t = 2 * tt + b
                    nc.tensor.matmul(
                        ps0[:],
                        ht_sb[:, t, :],
                        p2[:, b, 0:HALF],
                        start=(t == 0),
                        stop=(t == NT - 1),
                    )
                    nc.tensor.matmul(
                        ps1[:],
                        ht_sb[:, t, :],
                        p2[:, b, HALF:SLAB],
                        start=(t == 0),
                        stop=(t == NT - 1),
                    )

            # ---- epilogue: divide by denominator row, then ELU ----
            num = epool.tile([HT_F, SLAB], f32)
            nc.vector.tensor_copy(out=num[:, 0:HALF], in_=ps0[:])
            nc.vector.tensor_copy(out=num[:, HALF:SLAB], in_=ps1[:])

            # reciprocal_approx needs a partition-0 input; move the denominator
            # row there with an SBUF->SBUF DMA first.
            den_sb = epool.tile([1, SLAB], f32)
            nc.sync.dma_start(den_sb[:], num[OUT_F : OUT_F + 1, :])
            rcp = epool.tile([1, SLAB], f32)
            rcp_scratch = epool.tile([1, SLAB], f32)
            nc.vector.reciprocal_approx_accurate(
                out=rcp[:], in_=den_sb[:], scratch=rcp_scratch[:]
            )

            # broadcast rcp across 64 partitions via a K=1 matmul with ones
            ones = epool.tile([1, OUT_F], f32)
            nc.vector.memset(ones[:], 1.0)
            pb0 = pspool.tile([OUT_F, HALF], f32)
            pb1 = pspool.tile([OUT_F, HALF], f32)
            nc.tensor.matmul(pb0[:], ones[:], rcp[:, 0:HALF])
            nc.tensor.matmul(pb1[:], ones[:], rcp[:, HALF:SLAB])

            div = epool.tile([OUT_F, SLAB], f32)
            nc.vector.tensor_tensor(
                div[:, 0:HALF], num[0:OUT_F, 0:HALF], pb0[:], Alu.mult
            )
            nc.vector.tensor_tensor(
                div[:, HALF:SLAB], num[0:OUT_F, HALF:SLAB], pb1[:], Alu.mult
            )

            # elu(x) = relu(x) + min(exp(x) - 1, 0)
            ex = epool.tile([OUT_F, SLAB], f32)
            nc.scalar.activation(ex[:], div[:], mybir.ActivationFunctionType.Exp)
            exm = epool.tile([OUT_F, SLAB], f32)
            nc.vector.tensor_scalar(
                exm[:], ex[:], 1.0, 0.0, Alu.subtract, Alu.min
            )
            rl = epool.tile([OUT_F, SLAB], f32)
            nc.vector.tensor_scalar(rl[:], div[:], 0.0, None, Alu.max)
            ov = epool.tile([OUT_F, SLAB], f32)
            nc.vector.tensor_tensor(ov[:], exm[:], rl[:], Alu.add)

            nc.sync.dma_start(out[:], ov[:])

    nc.finalize()
    return nc


def _get_nc():
    global _nc_cache
    if _nc_cache is None:
        _nc_cache = _build_bass()
    return _nc_cache


def prepare_inputs(input, adj, W, a):
    """Host-side O(N*F) precompute + input marshaling. Returns per-core input
    maps for the SPMD bass kernel."""
    f32 = np.float32
    input = np.asarray(input, dtype=f32)
    W = np.asarray(W, dtype=f32)
    a = np.asarray(a, dtype=f32)
    adj = np.asarray(adj)

    h = input @ W  # [N, 64]
    f_src = h @ a[:OUT_F]
    f_dst = h @ a[OUT_F:]

    u = np.exp(0.8 * f_src).astype(_bf16)  # [N] per output row i
    v = np.exp(0.8 * f_dst).astype(f32)  # [N] per neighbor j
    q = np.exp(0.2 * f_dst).astype(f32)

    htil = np.empty((N, HT_F), f32)
    htil[:, :OUT_F] = h * q[:, None]
    htil[:, OUT_F] = q
    # device layout: partition p holds chunk t at columns [t*65, (t+1)*65)
    ht_dev = np.ascontiguousarray(
        htil.reshape(NT, P, HT_F).transpose(1, 0, 2).reshape(P, NT * HT_F)
    ).astype(_bf16)

    vT_dev = np.ascontiguousarray(v.reshape(NT, P).T)  # [128, 64] f32

    # mask, transposed to [j, i], as bf16 {0.0, 1.0} via bit pattern
    m16 = (adj.T != 0).astype(np.uint16)
    m16 *= np.uint16(0x3F80)  # bf16 bits of 1.0
    maskT = m16.view(_bf16)  # [N(j), N(i)]

    in_maps = []
    for c in range(N_CORES):
        sl = slice(c * SLAB, (c + 1) * SLAB)
        in_maps.append(
            {
                "maskT": np.ascontiguousarray(maskT[:, sl]),
                "u_bc": np.ascontiguousarray(
                    np.broadcast_to(u[sl][None, :], (P, SLAB))
                ),
                "vT": vT_dev,
                "ht": ht_dev,
            }
        )
    return in_maps


def assemble_output(results):
    """results: list of 8 dicts with 'out' [64, 1024] f32 -> [N, 64] f32."""
    hp = np.empty((N, OUT_F), np.float32)
    for c in range(N_CORES):
        hp[c * SLAB : (c + 1) * SLAB] = results[c]["out"].T
    return hp


def kernel(input, adj, W, a):
    from concourse.bass_utils import run_bass_kernel_spmd

    nc = _get_nc()
    in_maps = prepare_inputs(input, adj, W, a)
    res = run_bass_kernel_spmd(nc, in_maps, core_ids=list(range(N_CORES)))
    return assemble_output(res.results)


# revision 12
# speedup vs baseline: 1.2372x; 1.2372x over previous
"""GAT attention head (nn_AttHead) on 8 Trainium2 NeuronCores.

Reference computation:
    h = input @ W                  [N, F]
    e_ij = leakyrelu(f_src_i + f_dst_j, 0.2);  masked softmax over j (mask=adj)
    h' = elu(softmax(e) @ h)

Key restructuring used here (exact algebra, not an approximation):
    exp(lrelu(s)) = exp(0.2 s) * max(exp(0.8 s), 1)
    s_ij = f_src_i + f_dst_j is rank-1, so with
        u_i = exp(0.8 f_src_i), v_j = exp(0.8 f_dst_j), q_j = exp(0.2 f_dst_j)
    the row factor exp(0.2 f_src_i) cancels in the softmax and
        att_ij ∝ A_ij * q_j * max(u_i v_j, 1)
        h'_i = (Σ_j A_ij max(u_i v_j,1) [q_j h_j, q_j]) / (denominator column)
    This removes every transcendental from the O(N^2) inner loop: per tile the
    device only needs one tensor_scalar (mult+max), one tensor_tensor (mask
    multiply) and a matmul accumulation; u, v, q are O(N) host precomputes.

Sharding: row-parallel over the N=8192 output rows; core c owns rows
[c*1024, (c+1)*1024). Scores are built transposed ([j on partitions, i free])
so the PE can contract over j directly. The adjacency mask is shipped as a
bf16 {0,1} matrix transposed to [j, i] layout (host-side data marshaling).
"""

import numpy as np
import ml_dtypes

N = 8192
IN_F = 128
OUT_F = 64
HT_F = OUT_F + 1  # h-tilde carries a denominator ones-column (scaled by q)
N_CORES = 8
SLAB = N // N_CORES  # 1024 output rows per core
P = 128
NT = N // P  # 64 j-chunks of 128
HALF = SLAB // 2  # PSUM free-dim limit for fp32 output is 512

_bf16 = ml_dtypes.bfloat16

_nc_cache = None


def _build_bass():
    import concourse.mybir as mybir
    import concourse.tile as tile
    from concourse import bacc

    bf = mybir.dt.bfloat16
    f32 = mybir.dt.float32
    Alu = mybir.AluOpType

    nc = bacc.Bacc("TRN2", target_bir_lowering=False, debug=False)

    maskT = nc.dram_tensor("maskT", [N, SLAB], bf, kind="ExternalInput")
    u_bc = nc.dram_tensor("u_bc", [P, SLAB], bf, kind="ExternalInput")
    vT = nc.dram_tensor("vT", [P, NT], f32, kind="ExternalInput")
    ht = nc.dram_tensor("ht", [P, NT * HT_F], bf, kind="ExternalInput")
    out = nc.dram_tensor("out", [OUT_F, SLAB], f32, kind="ExternalOutput")

    maskT_t = maskT.rearrange("(t p) i -> t p i", p=P)

    with tile.TileContext(nc) as tc:
        with (
            tc.tile_pool(name="const", bufs=1) as cpool,
            tc.tile_pool(name="mask", bufs=8) as mpool,
            tc.tile_pool(name="pt", bufs=4) as ppool,
            tc.tile_pool(name="ps", bufs=1, space="PSUM") as pspool,
            tc.tile_pool(name="epi", bufs=1) as epool,
        ):
            u_sb = cpool.tile([P, SLAB], bf)
            nc.sync.dma_start(u_sb[:], u_bc[:])
            vT_sb = cpool.tile([P, NT], f32)
            nc.sync.dma_start(vT_sb[:], vT[:])
            ht_sb = cpool.tile([P, NT, HT_F], bf)
            nc.sync.dma_start(ht_sb[:], ht.rearrange("p (t f) -> p t f", f=HT_F))

            # Warm the ACT exp table during the main loop (ScalarE is idle);
            # output is unused.
            warm = cpool.tile([P, 8], f32)
            nc.scalar.activation(
                warm[:], u_sb[:, 0:8], mybir.ActivationFunctionType.Exp
            )

            ps0 = pspool.tile([HT_F, HALF], f32)
            ps1 = pspool.tile([HT_F, HALF], f32)

            # Four j-chunks per group: one 1MB mask DMA; per chunk a single
            # fused DVE op  p = max(u'_i, v_j) * mask  (the row factor u_i of
            # max(u_i v_j, 1) = u_i*max(v_j, 1/u_i) cancels in the softmax),
            # then 2 matmul accumulations.
            GRP = 4
            for tt in range(NT // GRP):
                m4 = mpool.tile([P, GRP, SLAB], bf)
                nc.sync.dma_start(
                    m4[:],
                    maskT_t[GRP * tt : GRP * (tt + 1)].rearrange("t p i -> p t i"),
                )
                p4 = ppool.tile([P, GRP, SLAB], bf)
                for b in range(GRP):
                    t = GRP * tt + b
                    nc.vector.scalar_tensor_tensor(
                        out=p4[:, b, :],
                        in0=u_sb[:],
                        scalar=vT_sb[:, t : t + 1],
                        in1=m4[:, b, :],
                        op0=Alu.max,
                        op1=Alu.mult,
                    )
                    nc.tensor.matmul(
                        ps0[:],
                        ht_sb[:, t, :],
                        p4[:, b, 0:HALF],
                        start=(t == 0),
                        stop=(t == NT - 1),
                    )
                    nc.tensor.matmul(
                        ps1[:],
                        ht_sb[:, t, :],
                        p4[:, b, HALF:SLAB],
                        start=(t == 0),
                        stop=(t == NT - 1),
                    )

            # ---- epilogue: divide by denominator row, then ELU ----
            num = epool.tile([HT_F, SLAB], f32)
            nc.vector.tensor_copy(out=num[:, 0:HALF], in_=ps0[:])
            nc.vector.tensor_copy(out=num[:, HALF:SLAB], in_=ps1[:])

            # reciprocal_approx needs a partition-0 input; move the denominator
            # row there with an SBUF->SBUF DMA first.
            den_sb = epool.tile([1, SLAB], f32)
            nc.sync.dma_start(den_sb[:], num[OUT_F : OUT_F + 1, :])
            rcp = epool.tile([1, SLAB], f32)
            rcp_scratch = epool.tile([1, SLAB], f32)
            nc.vector.reciprocal_approx_accurate(
                out=rcp[:], in_=den_sb[:], scratch=rcp_scratch[:]
            )

            # broadcast rcp across 64 partitions via a K=1 matmul with ones
            ones = epool.tile([1, OUT_F], f32)
            nc.vector.memset(ones[:], 1.0)
            pb0 = pspool.tile([OUT_F, HALF], f32)
            pb1 = pspool.tile([OUT_F, HALF], f32)
            nc.tensor.matmul(pb0[:], ones[:], rcp[:, 0:HALF])
            nc.tensor.matmul(pb1[:], ones[:], rcp[:, HALF:SLAB])

            div = epool.tile([OUT_F, SLAB], f32)
            nc.vector.tensor_tensor(
                div[:, 0:HALF], num[0:OUT_F, 0:HALF], pb0[:], Alu.mult
            )
            nc.vector.tensor_tensor(
                div[:, HALF:SLAB], num[0:OUT_F, HALF:SLAB], pb1[:], Alu.mult
            )

            # elu(x) = relu(x) + min(exp(x) - 1, 0)
            ex = epool.tile([OUT_F, SLAB], f32)
            nc.scalar.activation(ex[:], div[:], mybir.ActivationFunctionType.Exp)
            exm = epool.tile([OUT_F, SLAB], f32)
            nc.vector.tensor_scalar(
                exm[:], ex[:], 1.0, 0.0, Alu.subtract, Alu.min
            )
            rl = epool.tile([OUT_F, SLAB], f32)
            nc.vector.tensor_scalar(rl[:], div[:], 0.0, None, Alu.max)
            ov = epool.tile([OUT_F, SLAB], f32)
            nc.vector.tensor_tensor(ov[:], exm[:], rl[:], Alu.add)

            nc.sync.dma_start(out[:], ov[:])

    nc.finalize()
    return nc


def _get_nc():
    global _nc_cache
    if _nc_cache is None:
        _nc_cache = _build_bass()
    return _nc_cache


def prepare_inputs(input, adj, W, a):
    """Host-side O(N*F) precompute + input marshaling. Returns per-core input
    maps for the SPMD bass kernel."""
    f32 = np.float32
    input = np.asarray(input, dtype=f32)
    W = np.asarray(W, dtype=f32)
    a = np.asarray(a, dtype=f32)
    adj = np.asarray(adj)

    h = input @ W  # [N, 64]
    f_src = h @ a[:OUT_F]
    f_dst = h @ a[OUT_F:]

    u = np.exp(-0.8 * f_src).astype(_bf16)  # u' = exp(-0.8 f_src) per row i
    v = np.exp(0.8 * f_dst).astype(f32)  # [N] per neighbor j
    q = np.exp(0.2 * f_dst).astype(f32)

    htil = np.empty((N, HT_F), f32)
    htil[:, :OUT_F] = h * q[:, None]
    htil[:, OUT_F] = q
    # device layout: partition p holds chunk t at columns [t*65, (t+1)*65)
    ht_dev = np.ascontiguousarray(
        htil.reshape(NT, P, HT_F).transpose(1, 0, 2).reshape(P, NT * HT_F)
    ).astype(_bf16)

    vT_dev = np.ascontiguousarray(v.reshape(NT, P).T)  # [128, 64] f32

    # mask, transposed to [j, i], as bf16 {0.0, 1.0} via bit pattern
    m16 = (adj.T != 0).astype(np.uint16)
    m16 *= np.uint16(0x3F80)  # bf16 bits of 1.0
    maskT = m16.view(_bf16)  # [N(j), N(i)]

    in_maps = []
    for c in range(N_CORES):
        sl = slice(c * SLAB, (c + 1) * SLAB)
        in_maps.append(
            {
                "maskT": np.ascontiguousarray(maskT[:, sl]),
                "u_bc": np.ascontiguousarray(
                    np.broadcast_to(u[sl][None, :], (P, SLAB))
                ),
                "vT": vT_dev,
                "ht": ht_dev,
            }
        )
    return in_maps


def assemble_output(results):
    """results: list of 8 dicts with 'out' [64, 1024] f32 -> [N, 64] f32."""
    hp = np.empty((N, OUT_F), np.float32)
    for c in range(N_CORES):
        hp[c * SLAB : (c + 1) * SLAB] = results[c]["out"].T
    return hp


def kernel(input, adj, W, a):
    from concourse.bass_utils import run_bass_kernel_spmd

    nc = _get_nc()
    in_maps = prepare_inputs(input, adj, W, a)
    res = run_bass_kernel_spmd(nc, in_maps, core_ids=list(range(N_CORES)))
    return assemble_output(res.results)


# revision 14
# speedup vs baseline: 1.7645x; 1.4262x over previous
"""GAT attention head (nn_AttHead) on 8 Trainium2 NeuronCores.

Reference computation:
    h = input @ W                  [N, F]
    e_ij = leakyrelu(f_src_i + f_dst_j, 0.2);  masked softmax over j (mask=adj)
    h' = elu(softmax(e) @ h)

Key restructuring used here (exact algebra, not an approximation):
    exp(lrelu(s)) = exp(0.2 s) * max(exp(0.8 s), 1)
    s_ij = f_src_i + f_dst_j is rank-1, so with
        u_i = exp(0.8 f_src_i), v_j = exp(0.8 f_dst_j), q_j = exp(0.2 f_dst_j)
    the row factor exp(0.2 f_src_i) cancels in the softmax and
        att_ij ∝ A_ij * q_j * max(u_i v_j, 1)
        h'_i = (Σ_j A_ij max(u_i v_j,1) [q_j h_j, q_j]) / (denominator column)
    This removes every transcendental from the O(N^2) inner loop: per tile the
    device only needs one tensor_scalar (mult+max), one tensor_tensor (mask
    multiply) and a matmul accumulation; u, v, q are O(N) host precomputes.

Sharding: row-parallel over the N=8192 output rows; core c owns rows
[c*1024, (c+1)*1024). Scores are built transposed ([j on partitions, i free])
so the PE can contract over j directly. The adjacency mask is shipped as a
bf16 {0,1} matrix transposed to [j, i] layout (host-side data marshaling).
"""

import numpy as np
import ml_dtypes

N = 8192
IN_F = 128
OUT_F = 64
HT_F = OUT_F + 1  # h-tilde carries a denominator ones-column (scaled by q)
N_CORES = 8
SLAB = N // N_CORES  # 1024 output rows per core
P = 128
NT = N // P  # 64 j-chunks of 128
HALF = SLAB // 2  # PSUM free-dim limit for fp32 output is 512

_bf16 = ml_dtypes.bfloat16

_nc_cache = None


def _build_bass():
    import concourse.mybir as mybir
    import concourse.tile as tile
    from concourse import bacc

    bf = mybir.dt.bfloat16
    f32 = mybir.dt.float32
    Alu = mybir.AluOpType

    nc = bacc.Bacc("TRN2", target_bir_lowering=False, debug=False)

    maskT = nc.dram_tensor("maskT", [N, SLAB], bf, kind="ExternalInput")
    u_bc = nc.dram_tensor("u_bc", [P, SLAB], bf, kind="ExternalInput")
    vT = nc.dram_tensor("vT", [P, NT], f32, kind="ExternalInput")
    ht = nc.dram_tensor("ht", [P, NT * HT_F], bf, kind="ExternalInput")
    out = nc.dram_tensor("out", [OUT_F, SLAB], f32, kind="ExternalOutput")

    maskT_t = maskT.rearrange("(t p) i -> t p i", p=P)

    with tile.TileContext(nc) as tc:
        with (
            tc.tile_pool(name="const", bufs=1) as cpool,
            tc.tile_pool(name="mask", bufs=4) as mpool,
            tc.tile_pool(name="gt", bufs=4) as gpool,
            tc.tile_pool(name="pt", bufs=4) as ppool,
            tc.tile_pool(name="ps", bufs=1, space="PSUM") as pspool,
            tc.tile_pool(name="epi", bufs=1) as epool,
        ):
            u_sb = cpool.tile([P, SLAB], bf)
            nc.sync.dma_start(u_sb[:], u_bc[:])
            vT_sb = cpool.tile([P, NT], f32)
            nc.sync.dma_start(vT_sb[:], vT[:])
            ht_sb = cpool.tile([P, NT, HT_F], bf)
            nc.sync.dma_start(ht_sb[:], ht.rearrange("p (t f) -> p t f", f=HT_F))

            # Warm the ACT exp table during the main loop (ScalarE is idle);
            # output is unused.
            warm = cpool.tile([P, 8], f32)
            nc.scalar.activation(
                warm[:], u_sb[:, 0:8], mybir.ActivationFunctionType.Exp
            )

            ps0 = pspool.tile([HT_F, HALF], f32)
            ps1 = pspool.tile([HT_F, HALF], f32)

            # Four j-chunks per group: one 1MB mask DMA; per chunk a single
            # fused DVE op  p = max(u'_i, v_j) * mask  (the row factor u_i of
            # max(u_i v_j, 1) = u_i*max(v_j, 1/u_i) cancels in the softmax),
            # then 2 matmul accumulations.
            GRP = 4
            for tt in range(NT // GRP):
                m4 = mpool.tile([P, GRP, SLAB], bf)
                nc.sync.dma_start(
                    m4[:],
                    maskT_t[GRP * tt : GRP * (tt + 1)].rearrange("t p i -> p t i"),
                )
                g4 = gpool.tile([P, GRP, SLAB], bf)
                for b in range(GRP):
                    t = GRP * tt + b
                    # g = max(u'_i, v_j)
                    nc.vector.tensor_scalar(
                        g4[:, b, :], u_sb[:], vT_sb[:, t : t + 1], None, Alu.max
                    )
                p4 = ppool.tile([P, GRP, SLAB], bf)
                nc.vector.tensor_tensor(p4[:], g4[:], m4[:], Alu.mult)
                for b in range(GRP):
                    t = GRP * tt + b
                    nc.tensor.matmul(
                        ps0[:],
                        ht_sb[:, t, :],
                        p4[:, b, 0:HALF],
                        start=(t == 0),
                        stop=(t == NT - 1),
                    )
                    nc.tensor.matmul(
                        ps1[:],
                        ht_sb[:, t, :],
                        p4[:, b, HALF:SLAB],
                        start=(t == 0),
                        stop=(t == NT - 1),
                    )

            # ---- epilogue: divide by denominator row, then ELU ----
            num = epool.tile([HT_F, SLAB], f32)
            nc.vector.tensor_copy(out=num[:, 0:HALF], in_=ps0[:])
            nc.vector.tensor_copy(out=num[:, HALF:SLAB], in_=ps1[:])

            # reciprocal_approx needs a partition-0 input; move the denominator
            # row there with an SBUF->SBUF DMA first.
            den_sb = epool.tile([1, SLAB], f32)
            nc.sync.dma_start(den_sb[:], num[OUT_F : OUT_F + 1, :])
            rcp = epool.tile([1, SLAB], f32)
            rcp_scratch = epool.tile([1, SLAB], f32)
            nc.vector.reciprocal_approx_accurate(
                out=rcp[:], in_=den_sb[:], scratch=rcp_scratch[:]
            )

            # broadcast rcp across 64 partitions via a K=1 matmul with ones
            ones = epool.tile([1, OUT_F], f32)
            nc.vector.memset(ones[:], 1.0)
            pb0 = pspool.tile([OUT_F, HALF], f32)
            pb1 = pspool.tile([OUT_F, HALF], f32)
            nc.tensor.matmul(pb0[:], ones[:], rcp[:, 0:HALF])
            nc.tensor.matmul(pb1[:], ones[:], rcp[:, HALF:SLAB])

            div = epool.tile([OUT_F, SLAB], f32)
            nc.vector.tensor_tensor(
                div[:, 0:HALF], num[0:OUT_F, 0:HALF], pb0[:], Alu.mult
            )
            nc.vector.tensor_tensor(
                div[:, HALF:SLAB], num[0:OUT_F, HALF:SLAB], pb1[:], Alu.mult
            )

            # elu(x) = relu(x) + min(exp(x) - 1, 0)
            ex = epool.tile([OUT_F, SLAB], f32)
            nc.scalar.activation(ex[:], div[:], mybir.ActivationFunctionType.Exp)
            exm = epool.tile([OUT_F, SLAB], f32)
            nc.vector.tensor_scalar(
                exm[:], ex[:], 1.0, 0.0, Alu.subtract, Alu.min
            )
            rl = epool.tile([OUT_F, SLAB], f32)
            nc.vector.tensor_scalar(rl[:], div[:], 0.0, None, Alu.max)
            ov = epool.tile([OUT_F, SLAB], f32)
            nc.vector.tensor_tensor(ov[:], exm[:], rl[:], Alu.add)

            nc.sync.dma_start(out[:], ov[:])

    nc.finalize()
    return nc


def _get_nc():
    global _nc_cache
    if _nc_cache is None:
        _nc_cache = _build_bass()
    return _nc_cache


def prepare_inputs(input, adj, W, a):
    """Host-side O(N*F) precompute + input marshaling. Returns per-core input
    maps for the SPMD bass kernel."""
    f32 = np.float32
    input = np.asarray(input, dtype=f32)
    W = np.asarray(W, dtype=f32)
    a = np.asarray(a, dtype=f32)
    adj = np.asarray(adj)

    h = input @ W  # [N, 64]
    f_src = h @ a[:OUT_F]
    f_dst = h @ a[OUT_F:]

    u = np.exp(-0.8 * f_src).astype(_bf16)  # u' = exp(-0.8 f_src) per row i
    v = np.exp(0.8 * f_dst).astype(f32)  # [N] per neighbor j
    q = np.exp(0.2 * f_dst).astype(f32)

    htil = np.empty((N, HT_F), f32)
    htil[:, :OUT_F] = h * q[:, None]
    htil[:, OUT_F] = q
    # device layout: partition p holds chunk t at columns [t*65, (t+1)*65)
    ht_dev = np.ascontiguousarray(
        htil.reshape(NT, P, HT_F).transpose(1, 0, 2).reshape(P, NT * HT_F)
    ).astype(_bf16)

    vT_dev = np.ascontiguousarray(v.reshape(NT, P).T)  # [128, 64] f32

    # mask, transposed to [j, i], as bf16 {0.0, 1.0} via bit pattern
    m16 = (adj.T != 0).astype(np.uint16)
    m16 *= np.uint16(0x3F80)  # bf16 bits of 1.0
    maskT = m16.view(_bf16)  # [N(j), N(i)]

    in_maps = []
    for c in range(N_CORES):
        sl = slice(c * SLAB, (c + 1) * SLAB)
        in_maps.append(
            {
                "maskT": np.ascontiguousarray(maskT[:, sl]),
                "u_bc": np.ascontiguousarray(
                    np.broadcast_to(u[sl][None, :], (P, SLAB))
                ),
                "vT": vT_dev,
                "ht": ht_dev,
            }
        )
    return in_maps


def assemble_output(results):
    """results: list of 8 dicts with 'out' [64, 1024] f32 -> [N, 64] f32."""
    hp = np.empty((N, OUT_F), np.float32)
    for c in range(N_CORES):
        hp[c * SLAB : (c + 1) * SLAB] = results[c]["out"].T
    return hp


def kernel(input, adj, W, a):
    from concourse.bass_utils import run_bass_kernel_spmd

    nc = _get_nc()
    in_maps = prepare_inputs(input, adj, W, a)
    res = run_bass_kernel_spmd(nc, in_maps, core_ids=list(range(N_CORES)))
    return assemble_output(res.results)
